# revision 12
# baseline (speedup 1.0000x reference)
import numpy as np
import ml_dtypes
import orjson

import concourse.bass as bass
import concourse.tile as tile
from concourse import mybir
import concourse.bass2jax as bass2jax
from concourse.bass_utils import run_bass_kernel_spmd

BF16 = ml_dtypes.bfloat16
JITTER = 0.01
T, H, F, E = 512, 1024, 4096, 8
CAP = 152          # padded tokens per expert per dispatch (max observed 145)
NF = F // 128      # 32 f-tiles
NK = H // 128      # 8 k-tiles
G2DEPTH = 2        # f-tiles of deferral before the w2 matmul


# ---------------------------------------------------------------------------
# walrus codegen rejects >1 sem wait on ANY instruction; Tile's scheduler
# freely assigns several. Post-process the final BIR (after all fuse passes)
# to hoist extra waits onto single-wait carrier instructions inserted just
# before the original on the same engine.
# ---------------------------------------------------------------------------
_MAX_WAITS = 1


def _split_multiwait(bir):
    n_clones = 0
    for fn in bir.get("functions", []):
        for blk in fn.get("blocks", []):
            out = []
            for inst in blk["instructions"]:
                si = inst.get("sync_info")
                if si and len(si.get("on_wait") or []) > _MAX_WAITS:
                    waits = si["on_wait"]
                    keep, rest = waits[-_MAX_WAITS:], waits[:-_MAX_WAITS]
                    for w in rest:
                        n_clones += 1
                        out.append({
                            "debug": inst.get("debug"),
                            "engine": inst["engine"],
                            "ins": [],
                            "is_reset_sema": False,
                            "name": f"{inst['name']}-w{n_clones}",
                            "opcode": "Drain",
                            "outs": [],
                            "sync_info": {"on_update": [],
                                          "on_wait": [w]},
                        })
                    si["on_wait"] = keep
                out.append(inst)
            blk["instructions"] = out
    return n_clones


def _install_bir_patch():
    if getattr(bass2jax.compile_bir_kernel, "_mw_patch", False):
        return
    _orig = bass2jax.compile_bir_kernel

    def _patched(bir_json, tmpdir, neff_name="file.neff"):
        bir = orjson.loads(bir_json)
        if _split_multiwait(bir):
            bir_json = orjson.dumps(bir)
        return _orig(bir_json, tmpdir, neff_name=neff_name)

    _patched._mw_patch = True
    bass2jax.compile_bir_kernel = _patched


# ---------------------------------------------------------------------------
# Host-side routing: exact fp32 replica of the reference phi_routing.
# ---------------------------------------------------------------------------
def _softmax_f32(logits):
    m = np.max(logits, axis=-1, keepdims=True)
    e = np.exp(logits - m, dtype=np.float32)
    return e / np.sum(e, axis=-1, keepdims=True)


def _routing(x, gate_w):
    logits = (x @ gate_w.T).astype(np.float32)          # [T,E]
    t_idx = np.arange(logits.shape[0])
    sel1 = np.argmax(logits, axis=1)
    m1 = logits[t_idx, sel1][:, None]
    factor1 = np.maximum(np.abs(logits), m1)
    mask1 = (m1 - logits) / factor1 > np.float32(2.0 * JITTER)
    p1 = _softmax_f32(np.where(mask1, -np.inf, logits).astype(np.float32))
    mult1 = p1[t_idx, sel1]

    l2 = logits.copy()
    l2[t_idx, sel1] = -np.inf
    sel2 = np.argmax(l2, axis=1)
    m2 = logits[t_idx, sel2][:, None]
    factor2 = np.maximum(np.abs(logits), m2)
    mask2 = (m2 - logits) / factor2 > np.float32(2.0 * JITTER)
    drop1 = np.zeros_like(mask2)
    drop1[t_idx, sel1] = True
    p2 = _softmax_f32(np.where(mask2 | drop1, -np.inf, logits).astype(np.float32))
    mult2 = p2[t_idx, sel2]
    return sel1, sel2, mult1.astype(np.float32), mult2.astype(np.float32)


# ---------------------------------------------------------------------------
# Device program: one expert per core, tokens padded to CAP.
#   xt   [128, NK*CAP] bf16   x[idx].T grouped by k-tile
#   wall [F, 3*H]      bf16   per f-tile rows: [w1tT | w3tT | w2T]
#   y    [CAP, H]      f32    (h @ w2.T) unweighted
# ---------------------------------------------------------------------------
_PROGRAM = None


def _build_program():
    global _PROGRAM
    if _PROGRAM is not None:
        return _PROGRAM
    _install_bir_patch()
    nc = bass.Bass()
    bf = mybir.dt.bfloat16
    f32 = mybir.dt.float32
    xt = nc.declare_dram_parameter("xt", [128, NK * CAP], bf, isOutput=False)
    wall = nc.declare_dram_parameter("wall", [F, 3 * H], bf, isOutput=False)
    y = nc.declare_dram_parameter("y", [CAP, H], f32, isOutput=True)
    mchunks = [(0, 128), (128, CAP - 128)] if CAP > 128 else [(0, CAP)]

    with tile.TileContext(nc) as tc:
        with (tc.tile_pool(name="xp", bufs=1) as xp,
              tc.tile_pool(name="wp", bufs=4) as wp,
              tc.tile_pool(name="ap", bufs=4) as ap,
              tc.tile_pool(name="yo", bufs=1) as yo,
              tc.tile_pool(name="ps", bufs=2, space="PSUM") as ps,
              tc.tile_pool(name="py", bufs=1, space="PSUM") as py):
            xtile = xp.tile([128, NK * CAP], bf)
            for k in range(NK):
                nc.scalar.dma_start(out=xtile[:, k * CAP:(k + 1) * CAP],
                                    in_=xt[:, k * CAP:(k + 1) * CAP])
            psum_y = [[py.tile([128, 512], f32, name=f"psum_y_{m}_{n}")
                       for n in range(2)] for m in range(len(mchunks))]

            def emit_g2(av_p, wc_p, f_p):
                for m, (m0, mw) in enumerate(mchunks):
                    for n in range(2):
                        nc.tensor.matmul(
                            psum_y[m][n][0:mw, :],
                            lhsT=av_p[:, m0:m0 + mw],
                            rhs=wc_p[:, 2 * H + n * 512:2 * H + (n + 1) * 512],
                            start=(f_p == 0), stop=(f_p == NF - 1))

            pend = []  # deferred G2 work: (av, wc, f)
            for f in range(NF):
                wc = wp.tile([128, 3 * H], bf)
                half = 3 * H // 2
                nc.gpsimd.dma_start(out=wc[:, :half],
                                    in_=wall[f * 128:(f + 1) * 128, :half])
                nc.sync.dma_start(out=wc[:, half:],
                                  in_=wall[f * 128:(f + 1) * 128, half:])
                p1 = ps.tile([128, CAP], f32)
                p3 = ps.tile([128, CAP], f32)
                for k in range(NK):
                    nc.tensor.matmul(p1[:], lhsT=wc[:, k * 128:(k + 1) * 128],
                                     rhs=xtile[:, k * CAP:(k + 1) * CAP],
                                     start=(k == 0), stop=(k == NK - 1))
                if len(pend) >= G2DEPTH:
                    emit_g2(*pend.pop(0))
                for k in range(NK):
                    nc.tensor.matmul(p3[:], lhsT=wc[:, H + k * 128:H + (k + 1) * 128],
                                     rhs=xtile[:, k * CAP:(k + 1) * CAP],
                                     start=(k == 0), stop=(k == NK - 1))
                s1 = ap.tile([128, CAP], bf)
                nc.scalar.activation(s1[:], p1[:],
                                     mybir.ActivationFunctionType.Silu)
                av = ap.tile([128, CAP], bf)
                nc.vector.tensor_mul(av[:], s1[:], p3[:])
                pend.append((av, wc, f))
            for entry in pend:
                emit_g2(*entry)
            for m, (m0, mw) in enumerate(mchunks):
                for n in range(2):
                    yt = yo.tile([128, 512], f32, name=f"yt_{m}_{n}")
                    if n == 0:
                        nc.vector.tensor_copy(yt[0:mw, :], psum_y[m][n][0:mw, :])
                        nc.gpsimd.dma_start(
                            out=y[m0:m0 + mw, n * 512:(n + 1) * 512],
                            in_=yt[0:mw, :])
                    else:
                        nc.scalar.copy(yt[0:mw, :], psum_y[m][n][0:mw, :])
                        nc.sync.dma_start(
                            out=y[m0:m0 + mw, n * 512:(n + 1) * 512],
                            in_=yt[0:mw, :])
    _PROGRAM = nc
    return nc


# ---------------------------------------------------------------------------
# Host-side data marshalling
# ---------------------------------------------------------------------------
def _pack_weights(w1e, w2e, w3e):
    # w1t/w3t block f: [128 p(h in k), NK*128 (k,c f-in-tile)]
    w1b = np.ascontiguousarray(
        w1e.astype(BF16).reshape(NF, 128, NK, 128).transpose(0, 3, 2, 1))
    w3b = np.ascontiguousarray(
        w3e.astype(BF16).reshape(NF, 128, NK, 128).transpose(0, 3, 2, 1))
    w2b = np.ascontiguousarray(w2e.astype(BF16).T).reshape(NF, 128, H)
    return np.concatenate(
        [w1b.reshape(NF, 128, H), w3b.reshape(NF, 128, H), w2b],
        axis=2).reshape(F, 3 * H)


def _pack_tokens(x_bf, idx):
    xg = np.zeros((CAP, H), dtype=BF16)
    xg[:len(idx)] = x_bf[idx]
    return np.ascontiguousarray(
        xg.reshape(CAP, NK, 128).transpose(2, 1, 0)).reshape(128, NK * CAP)


def kernel(hidden_states, gate_w, w1, w2, w3):
    B, S, _ = hidden_states.shape
    x = np.asarray(hidden_states, dtype=np.float32).reshape(-1, H)
    sel1, sel2, mult1, mult2 = _routing(x, np.asarray(gate_w, np.float32))

    idx_e, wgt_e = [], []
    for e in range(E):
        idx = np.where((sel1 == e) | (sel2 == e))[0]
        idx_e.append(idx)
        wgt_e.append(np.where(sel1[idx] == e, mult1[idx], mult2[idx]))

    nc = _build_program()
    x_bf = x.astype(BF16)
    walls = [_pack_weights(w1[e], w2[e], w3[e]) for e in range(E)]

    n_runs = max(1, max((len(i) + CAP - 1) // CAP for i in idx_e))
    out = np.zeros((T, H), dtype=np.float32)
    for r in range(n_runs):
        in_maps = []
        chunks = []
        for e in range(E):
            chunk = idx_e[e][r * CAP:(r + 1) * CAP]
            chunks.append(chunk)
            in_maps.append({"xt": _pack_tokens(x_bf, chunk), "wall": walls[e]})
        res = run_bass_kernel_spmd(nc, in_maps, core_ids=list(range(E)))
        for e in range(E):
            chunk = chunks[e]
            if len(chunk) == 0:
                continue
            w = wgt_e[e][r * CAP:(r + 1) * CAP]
            out[chunk] += w[:, None] * res.results[e]["y"][:len(chunk)]
    return out.reshape(B, S, H)


# revision 35
# speedup vs baseline: 1.2842x; 1.2842x over previous
import numpy as np
import ml_dtypes
import orjson

import concourse.bass as bass
import concourse.tile as tile
from concourse import mybir
import concourse.bass2jax as bass2jax
from concourse.bass_utils import run_bass_kernel_spmd

BF16 = ml_dtypes.bfloat16
JITTER = 0.01
T, H, F, E = 512, 1024, 4096, 8
CAP = 145          # padded tokens per expert per dispatch (max observed 145;
                   # overflow falls back to extra dispatches in kernel())
NF = F // 128      # 32 f-tiles
NK = H // 128      # 8 k-tiles
G2DEPTH = 2        # f-tiles of deferral before the w2 matmul


# ---------------------------------------------------------------------------
# walrus codegen rejects >1 sem wait on ANY instruction; Tile's scheduler
# freely assigns several. Post-process the final BIR (after all fuse passes)
# to hoist extra waits onto single-wait carrier instructions inserted just
# before the original on the same engine.
# ---------------------------------------------------------------------------
_MAX_WAITS = 1


def _split_multiwait(bir):
    n_clones = 0
    for fn in bir.get("functions", []):
        for blk in fn.get("blocks", []):
            out = []
            for inst in blk["instructions"]:
                si = inst.get("sync_info")
                if si and len(si.get("on_wait") or []) > _MAX_WAITS:
                    waits = si["on_wait"]
                    keep, rest = waits[-_MAX_WAITS:], waits[:-_MAX_WAITS]
                    for w in rest:
                        n_clones += 1
                        out.append({
                            "debug": inst.get("debug"),
                            "engine": inst["engine"],
                            "ins": [],
                            "is_reset_sema": False,
                            "name": f"{inst['name']}-w{n_clones}",
                            "opcode": "Drain",
                            "outs": [],
                            "sync_info": {"on_update": [],
                                          "on_wait": [w]},
                        })
                    si["on_wait"] = keep
                out.append(inst)
            blk["instructions"] = out
    return n_clones


def _install_bir_patch():
    if getattr(bass2jax.compile_bir_kernel, "_mw_patch", False):
        return
    _orig = bass2jax.compile_bir_kernel

    def _patched(bir_json, tmpdir, neff_name="file.neff"):
        bir = orjson.loads(bir_json)
        if _split_multiwait(bir):
            bir_json = orjson.dumps(bir)
        return _orig(bir_json, tmpdir, neff_name=neff_name)

    _patched._mw_patch = True
    bass2jax.compile_bir_kernel = _patched


# ---------------------------------------------------------------------------
# Host-side routing: exact fp32 replica of the reference phi_routing.
# ---------------------------------------------------------------------------
def _softmax_f32(logits):
    m = np.max(logits, axis=-1, keepdims=True)
    e = np.exp(logits - m, dtype=np.float32)
    return e / np.sum(e, axis=-1, keepdims=True)


def _routing(x, gate_w):
    logits = (x @ gate_w.T).astype(np.float32)          # [T,E]
    t_idx = np.arange(logits.shape[0])
    sel1 = np.argmax(logits, axis=1)
    m1 = logits[t_idx, sel1][:, None]
    factor1 = np.maximum(np.abs(logits), m1)
    mask1 = (m1 - logits) / factor1 > np.float32(2.0 * JITTER)
    p1 = _softmax_f32(np.where(mask1, -np.inf, logits).astype(np.float32))
    mult1 = p1[t_idx, sel1]

    l2 = logits.copy()
    l2[t_idx, sel1] = -np.inf
    sel2 = np.argmax(l2, axis=1)
    m2 = logits[t_idx, sel2][:, None]
    factor2 = np.maximum(np.abs(logits), m2)
    mask2 = (m2 - logits) / factor2 > np.float32(2.0 * JITTER)
    drop1 = np.zeros_like(mask2)
    drop1[t_idx, sel1] = True
    p2 = _softmax_f32(np.where(mask2 | drop1, -np.inf, logits).astype(np.float32))
    mult2 = p2[t_idx, sel2]
    return sel1, sel2, mult1.astype(np.float32), mult2.astype(np.float32)


# ---------------------------------------------------------------------------
# Device program: one expert per core, tokens padded to CAP.
#   xt   [128, NK*CAP] bf16   x[idx].T grouped by k-tile
#   wall [F, 3*H]      bf16   per f-tile rows: [w1tT | w3tT | w2T]
#   y    [H, CAP]      f32    (h @ w2.T).T unweighted, token dim on free axis
# G2 keeps w2 stationary and av moving so the matmul cost is token-bound
# (N=CAP) instead of H-bound (N=512); output lands transposed.
# ---------------------------------------------------------------------------
_PROGRAM = None


def _build_program():
    global _PROGRAM
    if _PROGRAM is not None:
        return _PROGRAM
    _install_bir_patch()
    nc = bass.Bass()
    bf = mybir.dt.bfloat16
    f32 = mybir.dt.float32
    xt = nc.declare_dram_parameter("xt", [128, NK * CAP], bf, isOutput=False)
    wall = nc.declare_dram_parameter("wall", [F, 3 * H], bf, isOutput=False)
    y = nc.declare_dram_parameter("y", [H, CAP], f32, isOutput=True)

    with tile.TileContext(nc) as tc:
        with (tc.tile_pool(name="xp", bufs=1) as xp,
              tc.tile_pool(name="wp", bufs=6) as wp,
              tc.tile_pool(name="ap", bufs=4) as ap,
              tc.tile_pool(name="yo", bufs=1) as yo,
              tc.tile_pool(name="ps", bufs=3, space="PSUM") as ps,
              tc.tile_pool(name="py", bufs=1, space="PSUM") as py):
            xtile = xp.tile([128, NK * CAP], bf)
            for k in (0, 1, 2, 7):
                nc.scalar.dma_start(out=xtile[:, k * CAP:(k + 1) * CAP],
                                    in_=xt[:, k * CAP:(k + 1) * CAP])
            # warm the Act silu table off the critical path
            warm = ap.tile([128, 8], bf, name="warm")
            nc.vector.memset(warm[:], 0.0)
            nc.scalar.activation(warm[:], warm[:],
                                 mybir.ActivationFunctionType.Silu)
            # two h-chunk accumulators per 2KB psum bank (col 0:CAP, CAP:2CAP)
            psum_y = [py.tile([128, 512], f32, name=f"psum_y_{t}")
                      for t in range(4)]

            # one accumulation group per 2KB psum bank: start (zeroes the
            # whole bank) only on the bank's first write (f=0, even j),
            # stop only on its last (f=NF-1, odd j)
            def emit_g2(av_p, wc_p, f_p):
                for j in range(8):
                    nc.tensor.matmul(
                        psum_y[j // 2][:, (j % 2) * CAP:(j % 2) * CAP + CAP],
                        lhsT=wc_p[:, 2 * H + j * 128:2 * H + (j + 1) * 128],
                        rhs=av_p[:],
                        start=(f_p == 0 and j % 2 == 0),
                        stop=(f_p == NF - 1 and j % 2 == 1))

            pend = []  # deferred G2 work: (av, wc, f)
            for f in range(NF):
                wc = wp.tile([128, 3 * H], bf)
                rows = wall[f * 128:(f + 1) * 128, :]
                # w1 / w3 / w2 segments on separate queues; w2 is only
                # needed G2DEPTH iterations later, so Act can trail
                nc.gpsimd.dma_start(out=wc[:, :H], in_=rows[:, :H])
                nc.sync.dma_start(out=wc[:, H:2 * H], in_=rows[:, H:2 * H])
                nc.scalar.dma_start(out=wc[:, 2 * H:], in_=rows[:, 2 * H:])
                if f == 0:
                    for k in (3, 5):
                        nc.gpsimd.dma_start(
                            out=xtile[:, k * CAP:(k + 1) * CAP],
                            in_=xt[:, k * CAP:(k + 1) * CAP])
                    for k in (4, 6):
                        nc.sync.dma_start(
                            out=xtile[:, k * CAP:(k + 1) * CAP],
                            in_=xt[:, k * CAP:(k + 1) * CAP])
                p13 = ps.tile([128, 512], f32)
                p1 = p13[:, 0:CAP]
                p3 = p13[:, CAP:2 * CAP]
                # p1/p3 share a bank: single group spanning G1+G3 (start on
                # G1 k0 zeroes both halves, stop on G3 k7)
                for k in range(NK):
                    nc.tensor.matmul(p1[:], lhsT=wc[:, k * 128:(k + 1) * 128],
                                     rhs=xtile[:, k * CAP:(k + 1) * CAP],
                                     start=(k == 0), stop=False)
                last = f == NF - 1
                if len(pend) >= G2DEPTH and not last:
                    emit_g2(*pend.pop(0))
                for k in range(NK):
                    nc.tensor.matmul(p3[:], lhsT=wc[:, H + k * 128:H + (k + 1) * 128],
                                     rhs=xtile[:, k * CAP:(k + 1) * CAP],
                                     start=False, stop=(k == NK - 1))
                # last iteration: G3 stop first so the av chain (silu+mul)
                # overlaps the deferred G2 instead of stalling the tail
                if len(pend) >= G2DEPTH and last:
                    emit_g2(*pend.pop(0))
                s1 = ap.tile([128, CAP], bf)
                nc.scalar.activation(s1[:], p1[:],
                                     mybir.ActivationFunctionType.Silu)
                av = ap.tile([128, CAP], bf)
                nc.vector.tensor_mul(av[:], s1[:], p3[:])
                pend.append((av, wc, f))
            # interleave the remaining G2 groups per h-chunk so each psum
            # bank reaches its stop after len(pend) matmuls, not 8*len(pend)
            for j in range(8):
                for av_p, wc_p, f_p in pend:
                    nc.tensor.matmul(
                        psum_y[j // 2][:, (j % 2) * CAP:(j % 2) * CAP + CAP],
                        lhsT=wc_p[:, 2 * H + j * 128:2 * H + (j + 1) * 128],
                        rhs=av_p[:],
                        start=(f_p == 0 and j % 2 == 0),
                        stop=(f_p == NF - 1 and j % 2 == 1))
            yts = []
            for j in range(8):
                yt = yo.tile([128, CAP], f32, name=f"yt_{j}")
                src = psum_y[j // 2][:, (j % 2) * CAP:(j % 2) * CAP + CAP]
                if j % 2 == 0:
                    nc.vector.tensor_copy(yt[:], src)
                else:
                    nc.scalar.copy(yt[:], src)
                yts.append(yt)
            for j in range(8):
                eng = (nc.gpsimd, nc.sync, nc.scalar)[j % 3]
                eng.dma_start(out=y[j * 128:(j + 1) * 128, :], in_=yts[j][:])
    _PROGRAM = nc
    return nc


# ---------------------------------------------------------------------------
# Host-side data marshalling
# ---------------------------------------------------------------------------
def _pack_weights(w1e, w2e, w3e):
    # w1t/w3t block f: [128 p(h in k), NK*128 (k,c f-in-tile)]
    w1b = np.ascontiguousarray(
        w1e.astype(BF16).reshape(NF, 128, NK, 128).transpose(0, 3, 2, 1))
    w3b = np.ascontiguousarray(
        w3e.astype(BF16).reshape(NF, 128, NK, 128).transpose(0, 3, 2, 1))
    w2b = np.ascontiguousarray(w2e.astype(BF16).T).reshape(NF, 128, H)
    return np.concatenate(
        [w1b.reshape(NF, 128, H), w3b.reshape(NF, 128, H), w2b],
        axis=2).reshape(F, 3 * H)


def _pack_tokens(x_bf, idx):
    xg = np.zeros((CAP, H), dtype=BF16)
    xg[:len(idx)] = x_bf[idx]
    return np.ascontiguousarray(
        xg.reshape(CAP, NK, 128).transpose(2, 1, 0)).reshape(128, NK * CAP)


def kernel(hidden_states, gate_w, w1, w2, w3):
    B, S, _ = hidden_states.shape
    x = np.asarray(hidden_states, dtype=np.float32).reshape(-1, H)
    sel1, sel2, mult1, mult2 = _routing(x, np.asarray(gate_w, np.float32))

    idx_e, wgt_e = [], []
    for e in range(E):
        idx = np.where((sel1 == e) | (sel2 == e))[0]
        idx_e.append(idx)
        wgt_e.append(np.where(sel1[idx] == e, mult1[idx], mult2[idx]))

    nc = _build_program()
    x_bf = x.astype(BF16)
    walls = [_pack_weights(w1[e], w2[e], w3[e]) for e in range(E)]

    n_runs = max(1, max((len(i) + CAP - 1) // CAP for i in idx_e))
    out = np.zeros((T, H), dtype=np.float32)
    for r in range(n_runs):
        in_maps = []
        chunks = []
        for e in range(E):
            chunk = idx_e[e][r * CAP:(r + 1) * CAP]
            chunks.append(chunk)
            in_maps.append({"xt": _pack_tokens(x_bf, chunk), "wall": walls[e]})
        res = run_bass_kernel_spmd(nc, in_maps, core_ids=list(range(E)))
        for e in range(E):
            chunk = chunks[e]
            if len(chunk) == 0:
                continue
            w = wgt_e[e][r * CAP:(r + 1) * CAP]
            out[chunk] += w[:, None] * res.results[e]["y"][:, :len(chunk)].T
    return out.reshape(B, S, H)


# revision 40
# speedup vs baseline: 1.3078x; 1.0183x over previous
import numpy as np
import ml_dtypes
import orjson

import concourse.bass as bass
import concourse.tile as tile
from concourse import mybir
import concourse.bass2jax as bass2jax
from concourse.bass_utils import run_bass_kernel_spmd

BF16 = ml_dtypes.bfloat16
JITTER = 0.01
T, H, F, E = 512, 1024, 4096, 8
CAP = 145          # padded tokens per expert per dispatch (max observed 145;
                   # overflow falls back to extra dispatches in kernel())
NF = F // 128      # 32 f-tiles
NK = H // 128      # 8 k-tiles
G2DEPTH = 2        # f-tiles of deferral before the w2 matmul


# ---------------------------------------------------------------------------
# walrus codegen rejects >1 sem wait on ANY instruction; Tile's scheduler
# freely assigns several. Post-process the final BIR (after all fuse passes)
# to hoist extra waits onto single-wait carrier instructions inserted just
# before the original on the same engine.
# ---------------------------------------------------------------------------
_MAX_WAITS = 1


def _split_multiwait(bir):
    n_clones = 0
    for fn in bir.get("functions", []):
        for blk in fn.get("blocks", []):
            out = []
            for inst in blk["instructions"]:
                si = inst.get("sync_info")
                if si and len(si.get("on_wait") or []) > _MAX_WAITS:
                    waits = si["on_wait"]
                    keep, rest = waits[-_MAX_WAITS:], waits[:-_MAX_WAITS]
                    for w in rest:
                        n_clones += 1
                        out.append({
                            "debug": inst.get("debug"),
                            "engine": inst["engine"],
                            "ins": [],
                            "is_reset_sema": False,
                            "name": f"{inst['name']}-w{n_clones}",
                            "opcode": "Drain",
                            "outs": [],
                            "sync_info": {"on_update": [],
                                          "on_wait": [w]},
                        })
                    si["on_wait"] = keep
                out.append(inst)
            blk["instructions"] = out
    return n_clones


def _install_bir_patch():
    if getattr(bass2jax.compile_bir_kernel, "_mw_patch", False):
        return
    _orig = bass2jax.compile_bir_kernel

    def _patched(bir_json, tmpdir, neff_name="file.neff"):
        bir = orjson.loads(bir_json)
        if _split_multiwait(bir):
            bir_json = orjson.dumps(bir)
        return _orig(bir_json, tmpdir, neff_name=neff_name)

    _patched._mw_patch = True
    bass2jax.compile_bir_kernel = _patched


# ---------------------------------------------------------------------------
# Host-side routing: exact fp32 replica of the reference phi_routing.
# ---------------------------------------------------------------------------
def _softmax_f32(logits):
    m = np.max(logits, axis=-1, keepdims=True)
    e = np.exp(logits - m, dtype=np.float32)
    return e / np.sum(e, axis=-1, keepdims=True)


def _routing(x, gate_w):
    logits = (x @ gate_w.T).astype(np.float32)          # [T,E]
    t_idx = np.arange(logits.shape[0])
    sel1 = np.argmax(logits, axis=1)
    m1 = logits[t_idx, sel1][:, None]
    factor1 = np.maximum(np.abs(logits), m1)
    mask1 = (m1 - logits) / factor1 > np.float32(2.0 * JITTER)
    p1 = _softmax_f32(np.where(mask1, -np.inf, logits).astype(np.float32))
    mult1 = p1[t_idx, sel1]

    l2 = logits.copy()
    l2[t_idx, sel1] = -np.inf
    sel2 = np.argmax(l2, axis=1)
    m2 = logits[t_idx, sel2][:, None]
    factor2 = np.maximum(np.abs(logits), m2)
    mask2 = (m2 - logits) / factor2 > np.float32(2.0 * JITTER)
    drop1 = np.zeros_like(mask2)
    drop1[t_idx, sel1] = True
    p2 = _softmax_f32(np.where(mask2 | drop1, -np.inf, logits).astype(np.float32))
    mult2 = p2[t_idx, sel2]
    return sel1, sel2, mult1.astype(np.float32), mult2.astype(np.float32)


# ---------------------------------------------------------------------------
# Device program: one expert per core, tokens padded to CAP.
#   xt   [128, NK*CAP] bf16   x[idx].T grouped by k-tile
#   wall [F, 3*H]      bf16   per f-tile rows: [w1tT | w3tT | w2T]
#   y    [H, CAP]      f32    (h @ w2.T).T unweighted, token dim on free axis
# G2 keeps w2 stationary and av moving so the matmul cost is token-bound
# (N=CAP) instead of H-bound (N=512); output lands transposed.
# ---------------------------------------------------------------------------
_PROGRAM = None


def _build_program():
    global _PROGRAM
    if _PROGRAM is not None:
        return _PROGRAM
    _install_bir_patch()
    nc = bass.Bass()
    bf = mybir.dt.bfloat16
    f32 = mybir.dt.float32
    xt = nc.declare_dram_parameter("xt", [128, NK * CAP], bf, isOutput=False)
    wall = nc.declare_dram_parameter("wall", [F, 3 * H], bf, isOutput=False)
    # y[p, b*CAP + c] = out[b*128 + p, c]: one contiguous [128, 2*CAP] DMA
    # per psum bank; host un-permutes the h blocks
    y = nc.declare_dram_parameter("y", [128, 8 * CAP], f32, isOutput=True)

    with tile.TileContext(nc) as tc:
        with (tc.tile_pool(name="xp", bufs=1) as xp,
              tc.tile_pool(name="wp", bufs=6) as wp,
              tc.tile_pool(name="ap", bufs=4) as ap,
              tc.tile_pool(name="yo", bufs=1) as yo,
              tc.tile_pool(name="ps", bufs=3, space="PSUM") as ps,
              tc.tile_pool(name="py", bufs=1, space="PSUM") as py):
            xtile = xp.tile([128, NK * CAP], bf)
            for k in (0, 1, 2, 7):
                nc.scalar.dma_start(out=xtile[:, k * CAP:(k + 1) * CAP],
                                    in_=xt[:, k * CAP:(k + 1) * CAP])
            # warm the Act silu table off the critical path
            warm = ap.tile([128, 8], bf, name="warm")
            nc.vector.memset(warm[:], 0.0)
            nc.scalar.activation(warm[:], warm[:],
                                 mybir.ActivationFunctionType.Silu)
            # two h-chunk accumulators per 2KB psum bank (col 0:CAP, CAP:2CAP)
            psum_y = [py.tile([128, 512], f32, name=f"psum_y_{t}")
                      for t in range(4)]

            # one accumulation group per 2KB psum bank: start (zeroes the
            # whole bank) only on the bank's first write (f=0, even j),
            # stop only on its last (f=NF-1, odd j)
            def emit_g2(av_p, wc_p, f_p):
                for j in range(8):
                    nc.tensor.matmul(
                        psum_y[j // 2][:, (j % 2) * CAP:(j % 2) * CAP + CAP],
                        lhsT=wc_p[:, 2 * H + j * 128:2 * H + (j + 1) * 128],
                        rhs=av_p[:],
                        start=(f_p == 0 and j % 2 == 0),
                        stop=(f_p == NF - 1 and j % 2 == 1))

            pend = []  # deferred G2 work: (av, wc, f)
            for f in range(NF):
                wc = wp.tile([128, 3 * H], bf)
                rows = wall[f * 128:(f + 1) * 128, :]
                # w1 / w3 / w2 segments on separate queues; w2 is only
                # needed G2DEPTH iterations later, so Act can trail
                nc.gpsimd.dma_start(out=wc[:, :H], in_=rows[:, :H])
                nc.sync.dma_start(out=wc[:, H:2 * H], in_=rows[:, H:2 * H])
                nc.scalar.dma_start(out=wc[:, 2 * H:], in_=rows[:, 2 * H:])
                if f == 0:
                    for k in (3, 5):
                        nc.gpsimd.dma_start(
                            out=xtile[:, k * CAP:(k + 1) * CAP],
                            in_=xt[:, k * CAP:(k + 1) * CAP])
                    for k in (4, 6):
                        nc.sync.dma_start(
                            out=xtile[:, k * CAP:(k + 1) * CAP],
                            in_=xt[:, k * CAP:(k + 1) * CAP])
                p13 = ps.tile([128, 512], f32)
                p1 = p13[:, 0:CAP]
                p3 = p13[:, CAP:2 * CAP]
                # p1/p3 share a bank: single group spanning G1+G3 (start on
                # G1 k0 zeroes both halves, stop on G3 k7)
                for k in range(NK):
                    nc.tensor.matmul(p1[:], lhsT=wc[:, k * 128:(k + 1) * 128],
                                     rhs=xtile[:, k * CAP:(k + 1) * CAP],
                                     start=(k == 0), stop=False)
                last = f == NF - 1
                if len(pend) >= G2DEPTH and not last:
                    emit_g2(*pend.pop(0))
                for k in range(NK):
                    nc.tensor.matmul(p3[:], lhsT=wc[:, H + k * 128:H + (k + 1) * 128],
                                     rhs=xtile[:, k * CAP:(k + 1) * CAP],
                                     start=False, stop=(k == NK - 1))
                # last iteration: G3 stop first so the av chain (silu+mul)
                # overlaps the deferred G2 instead of stalling the tail
                if len(pend) >= G2DEPTH and last:
                    emit_g2(*pend.pop(0))
                s1 = ap.tile([128, CAP], bf)
                nc.scalar.activation(s1[:], p1[:],
                                     mybir.ActivationFunctionType.Silu)
                av = ap.tile([128, CAP], bf)
                nc.vector.tensor_mul(av[:], s1[:], p3[:])
                pend.append((av, wc, f))
            # interleave the remaining G2 groups per h-chunk so each psum
            # bank reaches its stop after len(pend) matmuls, not 8*len(pend)
            for j in range(8):
                for av_p, wc_p, f_p in pend:
                    nc.tensor.matmul(
                        psum_y[j // 2][:, (j % 2) * CAP:(j % 2) * CAP + CAP],
                        lhsT=wc_p[:, 2 * H + j * 128:2 * H + (j + 1) * 128],
                        rhs=av_p[:],
                        start=(f_p == 0 and j % 2 == 0),
                        stop=(f_p == NF - 1 and j % 2 == 1))
            yts = []
            for b in range(4):
                yt = yo.tile([128, 2 * CAP], f32, name=f"yt_{b}")
                src = psum_y[b][:, 0:2 * CAP]
                if b % 2 == 0:
                    nc.vector.tensor_copy(yt[:], src)
                else:
                    nc.scalar.copy(yt[:], src)
                yts.append(yt)
            for b in range(4):
                eng = (nc.sync, nc.scalar, nc.gpsimd, nc.sync)[b]
                eng.dma_start(out=y[:, 2 * b * CAP:2 * (b + 1) * CAP],
                              in_=yts[b][:])
    _PROGRAM = nc
    return nc


# ---------------------------------------------------------------------------
# Host-side data marshalling
# ---------------------------------------------------------------------------
def _pack_weights(w1e, w2e, w3e):
    # w1t/w3t block f: [128 p(h in k), NK*128 (k,c f-in-tile)]
    w1b = np.ascontiguousarray(
        w1e.astype(BF16).reshape(NF, 128, NK, 128).transpose(0, 3, 2, 1))
    w3b = np.ascontiguousarray(
        w3e.astype(BF16).reshape(NF, 128, NK, 128).transpose(0, 3, 2, 1))
    w2b = np.ascontiguousarray(w2e.astype(BF16).T).reshape(NF, 128, H)
    return np.concatenate(
        [w1b.reshape(NF, 128, H), w3b.reshape(NF, 128, H), w2b],
        axis=2).reshape(F, 3 * H)


def _pack_tokens(x_bf, idx):
    xg = np.zeros((CAP, H), dtype=BF16)
    xg[:len(idx)] = x_bf[idx]
    return np.ascontiguousarray(
        xg.reshape(CAP, NK, 128).transpose(2, 1, 0)).reshape(128, NK * CAP)


def kernel(hidden_states, gate_w, w1, w2, w3):
    B, S, _ = hidden_states.shape
    x = np.asarray(hidden_states, dtype=np.float32).reshape(-1, H)
    sel1, sel2, mult1, mult2 = _routing(x, np.asarray(gate_w, np.float32))

    idx_e, wgt_e = [], []
    for e in range(E):
        idx = np.where((sel1 == e) | (sel2 == e))[0]
        idx_e.append(idx)
        wgt_e.append(np.where(sel1[idx] == e, mult1[idx], mult2[idx]))

    nc = _build_program()
    x_bf = x.astype(BF16)
    walls = [_pack_weights(w1[e], w2[e], w3[e]) for e in range(E)]

    n_runs = max(1, max((len(i) + CAP - 1) // CAP for i in idx_e))
    out = np.zeros((T, H), dtype=np.float32)
    for r in range(n_runs):
        in_maps = []
        chunks = []
        for e in range(E):
            chunk = idx_e[e][r * CAP:(r + 1) * CAP]
            chunks.append(chunk)
            in_maps.append({"xt": _pack_tokens(x_bf, chunk), "wall": walls[e]})
        res = run_bass_kernel_spmd(nc, in_maps, core_ids=list(range(E)))
        for e in range(E):
            chunk = chunks[e]
            if len(chunk) == 0:
                continue
            w = wgt_e[e][r * CAP:(r + 1) * CAP]
            yf = res.results[e]["y"].reshape(128, 8, CAP) \
                .swapaxes(0, 1).reshape(H, CAP)
            out[chunk] += w[:, None] * yf[:, :len(chunk)].T
    return out.reshape(B, S, H)
